# revision 1
# baseline (speedup 1.0000x reference)
"""Dual-path multi-head attention on 8 trn2 NeuronCores.

Sharding: core c = (path p=c//4, batch b=c%4). Each core runs the full
pipeline for one path and one batch element: 3 input projections, 16-head
attention (S=1024, dh=64), output projection. No collectives.

Path 2 cross-wiring (q2 from k; k2,v2 from q) is handled purely by host-side
input routing - every core runs the identical SPMD program.

Device layouts (per core, all pre-packed on host for contiguous DMA runs):
  xq/xk/xv : [p, n, s]   = x.T blocked:  x[s, n*128+p]
  wq/wc    : [p, m, n, e'] = W[m*128+e', n*128+p]  (W.T blocked by out-block m)
  wv       : [p, n, e]   = Wv[e, n*128+p]
  Projections compute Q1T/K1T = [e, s] and V1 = [s, e]; scores are computed
  transposed (probs_T[sk, sq]) so softmax needs no transposes. Softmax is
  max-free (scores ~ N(0,1)); the denominator comes from a ones-column
  appended per head slot in v1e (PV yields [dh+1, sq], row 64 = sum).
Emission order software-pipelines Q/K projections of block m+1 between the
two attention heads of block m so their PSUM->SBUF drains hide under PE work.
"""

import numpy as np
import ml_dtypes

B, S, D, H, DH = 4, 1024, 1024, 16, 64
NB = D // 128  # 8 partition-blocks
HW = 65  # head slot width in v1e (64 data + 1 ones col)

_compiled = None


def _build():
    import concourse.bass as bass
    import concourse.mybir as mybir
    import concourse.tile as tile
    from concourse import bacc

    dt = mybir.dt
    f32, bf16, f32r = dt.float32, dt.bfloat16, dt.float32r

    nc = bacc.Bacc("TRN2", target_bir_lowering=False, debug=False)

    xq_d = nc.dram_tensor("xq", [128, NB, S], bf16, kind="ExternalInput")
    xk_d = nc.dram_tensor("xk", [128, NB, S], bf16, kind="ExternalInput")
    xv_d = nc.dram_tensor("xv", [128, NB, S], bf16, kind="ExternalInput")
    wq_d = nc.dram_tensor("wq", [128, NB, NB, 128], bf16, kind="ExternalInput")
    wk_d = nc.dram_tensor("wk", [128, NB, NB, 128], bf16, kind="ExternalInput")
    wv_d = nc.dram_tensor("wv", [128, NB, D], bf16, kind="ExternalInput")
    wc_d = nc.dram_tensor("wc", [128, NB, NB, 128], bf16, kind="ExternalInput")
    bq_d = nc.dram_tensor("bq", [128, NB], f32, kind="ExternalInput")
    bk_d = nc.dram_tensor("bk", [128, NB], f32, kind="ExternalInput")
    bc_d = nc.dram_tensor("bc", [128, NB], f32, kind="ExternalInput")
    bvB_d = nc.dram_tensor("bvB", [128, D], bf16, kind="ExternalInput")
    out_d = nc.dram_tensor("outT", [D, S], f32, kind="ExternalOutput")
    rdram = nc.dram_tensor("rbounce", [H, S], f32)

    ExpF = mybir.ActivationFunctionType.Exp

    with tile.TileContext(nc) as tc:
        with tc.tile_pool(name="x", bufs=3) as xp, \
             tc.tile_pool(name="wfull", bufs=1) as wfp, \
             tc.tile_pool(name="wblk", bufs=4) as wbp, \
             tc.tile_pool(name="cst", bufs=1) as cp, \
             tc.tile_pool(name="qk", bufs=4) as qkp, \
             tc.tile_pool(name="pers", bufs=1) as prp, \
             tc.tile_pool(name="pt", bufs=2) as ptp, \
             tc.tile_pool(name="stage", bufs=2) as stp, \
             tc.tile_pool(name="rcp", bufs=2) as rcp, \
             tc.tile_pool(name="ost", bufs=2) as ostp, \
             tc.tile_pool(name="mm", bufs=2, space="PSUM") as mmp, \
             tc.tile_pool(name="vp", bufs=2, space="PSUM") as vpp:

            # ---- loads: first V-proj blocks (interleaved for early start),
            # constants after the first block pair, then xq/xk; wc last.
            xv_t = xp.tile([128, NB, S], bf16, tag="x")
            wv_t = wfp.tile([128, NB, D], bf16)
            nc.sync.dma_start(out=xv_t[:, 0, :], in_=xv_d.ap()[:, 0, :])
            nc.sync.dma_start(out=wv_t[:, 0, :], in_=wv_d.ap()[:, 0, :])
            bq_t = cp.tile([128, NB], f32)
            nc.sync.dma_start(out=bq_t[:, :], in_=bq_d.ap())
            bk_t = cp.tile([128, NB], f32)
            nc.sync.dma_start(out=bk_t[:, :], in_=bk_d.ap())
            bc_t = cp.tile([128, NB], f32)
            nc.sync.dma_start(out=bc_t[:, :], in_=bc_d.ap())
            bvB_t = cp.tile([128, D], bf16)
            nc.sync.dma_start(out=bvB_t[:, :], in_=bvB_d.ap())
            ones64 = cp.tile([65, 64], bf16)
            nc.vector.memset(ones64[:, :], 1.0)
            for n in range(1, NB):
                nc.sync.dma_start(out=xv_t[:, n, :], in_=xv_d.ap()[:, n, :])
                nc.sync.dma_start(out=wv_t[:, n, :], in_=wv_d.ap()[:, n, :])
            xq_t = xp.tile([128, NB, S], bf16, tag="x")
            nc.sync.dma_start(out=xq_t[:, :, :], in_=xq_d.ap())
            xk_t = xp.tile([128, NB, S], bf16, tag="x")
            nc.sync.dma_start(out=xk_t[:, :, :], in_=xk_d.ap())

            v1e = prp.tile([128, NB, H * HW], bf16)
            a1 = [prp.tile([128, S], bf16, tag=f"a1_{n}", name=f"a1_{n}")
                  for n in range(NB)]

            # ones columns of v1e (softmax denominator trick)
            ones_ap = v1e[:, :, :].rearrange("p n (h x) -> p n h x", x=HW)[:, :, :, 64]
            nc.vector.memset(ones_ap, 1.0)

            def vproj_block(n2):
                ps = vpp.tile([128, 2, 512], f32, tag="vp", name=f"vps{n2}")
                for n in range(NB):
                    for c in range(2):
                        nc.tensor.matmul(
                            ps[:, c, :],
                            xv_t[:, n, n2 * 128:(n2 + 1) * 128],
                            wv_t[:, n, c * 512:(c + 1) * 512],
                            start=(n == 0), stop=(n == NB - 1),
                        )
                dst = v1e[:, n2, :].rearrange("p (c h x) -> p c h x", c=2, x=HW)[:, :, :, 0:64]
                ps_v = ps[:, :, :].rearrange("p c (h x) -> p c h x", x=64)
                bv_v = bvB_t[:, :].rearrange("p (c h x) -> p c h x", c=2, x=64)
                nc.vector.tensor_add(dst, ps_v, bv_v)

            def wblk_load(w_d, m):
                wb = wbp.tile([128, NB, 128], bf16, tag="wblk")
                nc.sync.dma_start(out=wb[:, :, :], in_=w_d.ap()[:, m, :, :])
                return wb

            def proj_block(wb, x_t, b_t, m):
                """[e-block m, s] = W.T-block @ x.T (+ bias) -> f32 tile.
                Kept in f32 so the scores matmuls can run in float32r
                (full-rate for moving dim >= 256) for better accuracy."""
                ps = vpp.tile([128, 2, 512], f32, tag="vp")
                for n in range(NB):
                    for c in range(2):
                        nc.tensor.matmul(
                            ps[:, c, :], wb[:, n, :], x_t[:, n, c * 512:(c + 1) * 512],
                            start=(n == 0), stop=(n == NB - 1),
                        )
                ob = qkp.tile([128, S], f32r, tag="qk")
                nc.vector.tensor_scalar_add(
                    ob[:, :].rearrange("p (c s) -> p c s", c=2), ps[:, :, :], b_t[:, m:m + 1])
                return ob

            def head(h, q1b, k1b, defer_norm=False, mid_cb=None):
                po = (h % 2) * 64
                pt = ptp.tile([128, NB, S], bf16, tag="pt")
                vps = vpp.tile([65, 2, 512], f32, tag="vp")

                def pv_chunk(n):
                    for c in range(2):
                        nc.tensor.matmul(
                            vps[:, c, :],
                            v1e[:, n, h * HW:(h + 1) * HW],
                            pt[:, n, c * 512:(c + 1) * 512],
                            start=(n == 0), stop=(n == NB - 1),
                        )

                # interleave PV fill in 2-chunk bursts between scores chunks
                # (fewer PE context switches than per-chunk interleave)
                for n in range(NB):
                    sps = mmp.tile([128, 2, 512], f32, tag="mm")
                    for c in range(2):
                        nc.tensor.matmul(
                            sps[:, c, :],
                            k1b[po:po + 64, n * 128:(n + 1) * 128],
                            q1b[po:po + 64, c * 512:(c + 1) * 512],
                            start=True, stop=True,
                        )
                    nc.scalar.activation(
                        out=pt[:, n, :].rearrange("p (c s) -> p c s", c=2),
                        in_=sps[:, :, :], func=ExpF, scale=0.125)
                    if n in (3, 5, 7):
                        pv_chunk(n - 3)
                        pv_chunk(n - 2)
                if mid_cb is not None:
                    mid_cb()
                for n in range(NB - 2, NB):
                    pv_chunk(n)

                if h < H - 1:
                    # DRAM-bounce partition broadcast of 1/denom (off critical
                    # path for all but the last heads)
                    rc = rcp.tile([65, S], f32, tag="rc")
                    nc.vector.reciprocal(
                        out=rc[64:65, :].rearrange("p (c s) -> p c s", c=2),
                        in_=vps[64:65, :, :])
                    nc.gpsimd.dma_start(out=rdram.ap()[h:h + 1, :], in_=rc[64:65, :])
                    rb = rcp.tile([64, S], f32, tag="rb")
                    nc.gpsimd.dma_start(
                        out=rb[:, :], in_=rdram.ap()[h:h + 1, :].to_broadcast((64, S)))
                else:
                    # last head pair feeds the output projection directly:
                    # use the shorter PE-broadcast chain (K=1 matmul) instead
                    rc = rcp.tile([65, S], f32, tag="rc")
                    nc.vector.reciprocal(
                        out=rc[64:65, :].rearrange("p (c s) -> p c s", c=2),
                        in_=vps[64:65, :, :])
                    rcb = rcp.tile([65, S], bf16, tag="rcb")
                    nc.vector.tensor_copy(rcb[64:65, :], rc[64:65, :])
                    rbp = mmp.tile([64, 2, 512], f32, tag="mm")
                    for c in range(2):
                        nc.tensor.matmul(
                            rbp[:, c, :], ones64[64:65, 0:64],
                            rcb[64:65, c * 512:(c + 1) * 512],
                            start=True, stop=True)
                    rb = rcp.tile([64, S], f32, tag="rb")
                    nc.vector.tensor_copy(
                        rb[:, :].rearrange("p (c s) -> p c s", c=2), rbp[:, :, :])

                m = h // 2

                def finish():
                    if h % 2 == 0:
                        dst = a1[m][0:64, :]
                        st = None
                    else:
                        st = stp.tile([64, S], bf16, tag="st")
                        dst = st[:, :]
                    nc.vector.tensor_mul(
                        dst.rearrange("p (c s) -> p c s", c=2),
                        vps[0:64, :, :],
                        rb[:, :].rearrange("p (c s) -> p c s", c=2))
                    if st is not None:
                        if h == H - 1:
                            nc.sync.dma_start(out=a1[m][64:128, :], in_=st[:, :])
                        else:
                            nc.gpsimd.dma_start(out=a1[m][64:128, :], in_=st[:, :])

                if defer_norm:
                    return finish
                finish()

            # ---- V projection (with Q0/K0 interleaved near the end so their
            # PSUM->SBUF drains hide under the remaining V-proj blocks) ----
            wqb = wblk_load(wq_d, 0)
            wkb = wblk_load(wk_d, 0)
            wc_t = wfp.tile([128, NB, NB, 128], bf16, tag="wc")
            nc.sync.dma_start(out=wc_t[:, :, :, :], in_=wc_d.ap())
            for n2 in range(NB - 2):
                vproj_block(n2)
            q1b = proj_block(wqb, xq_t, bq_t, 0)
            vproj_block(NB - 2)
            k1b = proj_block(wkb, xk_t, bk_t, 0)
            vproj_block(NB - 1)
            for m in range(NB):
                if m < NB - 1:
                    head(2 * m, q1b, k1b)
                    nwqb = wblk_load(wq_d, m + 1)
                    nwkb = wblk_load(wk_d, m + 1)
                    nq1b = proj_block(nwqb, xq_t, bq_t, m + 1)
                    nk1b = proj_block(nwkb, xk_t, bk_t, m + 1)
                    head(2 * m + 1, q1b, k1b)
                    q1b, k1b = nq1b, nk1b
                else:
                    # last pair: defer head-14's normalize multiply so its
                    # DRAM-bounce hides under head-15's scores, then finish it
                    # mid-head-15 (keeps the a1[7] tail chain short)
                    fin14 = head(2 * m, q1b, k1b, defer_norm=True)
                    head(2 * m + 1, q1b, k1b, mid_cb=fin14)

            # ---- output projection ----
            for m in range(NB):
                ops = mmp.tile([128, 2, 512], f32, tag="mm")
                for n in range(NB):
                    for c in range(2):
                        nc.tensor.matmul(
                            ops[:, c, :], wc_t[:, m, n, :], a1[n][:, c * 512:(c + 1) * 512],
                            start=(n == 0), stop=(n == NB - 1),
                        )
                if m < NB - 1:
                    ot = ostp.tile([128, 2, 512], f32, tag="ost")
                    nc.vector.tensor_scalar_add(ot[:, :, :], ops[:, :, :], bc_t[:, m:m + 1])
                    nc.sync.dma_start(
                        out=out_d.ap()[m * 128:(m + 1) * 128, :].rearrange(
                            "p (c s) -> p c s", c=2),
                        in_=ot[:, :, :])
                else:
                    # split the last store so its drain+DMA chain pipelines
                    for c in range(2):
                        ot = ostp.tile([128, 512], f32, tag="ostl")
                        nc.vector.tensor_scalar_add(ot[:, :], ops[:, c, :], bc_t[:, m:m + 1])
                        nc.sync.dma_start(
                            out=out_d.ap()[m * 128:(m + 1) * 128,
                                           c * 512:(c + 1) * 512],
                            in_=ot[:, :])

    nc.compile()
    return nc


def _get_nc():
    global _compiled
    if _compiled is None:
        _compiled = _build()
    return _compiled


def _make_in_maps(q, k, v, Wq, bq, Wk, bk, Wv, bv, Wq2, bq2, Wk2, bk2, Wv2, bv2,
                  Wc, bc, Wc2, bc2):
    bf16 = ml_dtypes.bfloat16

    def xpack(x):  # [s, d] -> [p, n, s]
        x = np.asarray(x, np.float32)
        return np.ascontiguousarray(x.reshape(S, NB, 128).transpose(2, 1, 0)).astype(bf16)

    def wpack(w):  # W[e, d] -> [p, m, n, e']
        w = np.asarray(w, np.float32)
        return np.ascontiguousarray(
            w.reshape(NB, 128, NB, 128).transpose(3, 0, 2, 1)).astype(bf16)

    def wvpack(w):  # Wv[e, d] -> [p, n, e]
        w = np.asarray(w, np.float32)
        return np.ascontiguousarray(w.T.reshape(NB, 128, D).transpose(1, 0, 2)).astype(bf16)

    def btile(b):
        return np.ascontiguousarray(np.asarray(b, np.float32).reshape(NB, 128).T)

    def brep(b):
        return np.ascontiguousarray(
            np.broadcast_to(np.asarray(b, np.float32), (128, D))).astype(bf16)

    paths = [
        dict(wq=wpack(Wq), wk=wpack(Wk), wv=wvpack(Wv), wc=wpack(Wc),
             bq=btile(bq), bk=btile(bk), bc=btile(bc), bvB=brep(bv)),
        dict(wq=wpack(Wq2), wk=wpack(Wk2), wv=wvpack(Wv2), wc=wpack(Wc2),
             bq=btile(bq2), bk=btile(bk2), bc=btile(bc2), bvB=brep(bv2)),
    ]
    in_maps = []
    for c in range(8):
        p, b = c // 4, c % 4
        if p == 0:
            xq, xk, xv = xpack(q[b]), xpack(k[b]), xpack(v[b])
        else:
            # path 2: q2 from k; k2, v2 from q
            xq, xk, xv = xpack(k[b]), xpack(q[b]), xpack(q[b])
        in_maps.append(dict(paths[p], xq=xq, xk=xk, xv=xv))
    return in_maps


def _run(in_maps, trace=False):
    from concourse.bass_utils import run_bass_kernel_spmd
    nc = _get_nc()
    return run_bass_kernel_spmd(nc, in_maps, core_ids=list(range(8)), trace=trace)


def kernel(**inputs):
    in_maps = _make_in_maps(**inputs)
    try:
        res = _run(in_maps)
    except Exception:
        # transient NRT_EXEC_UNIT_UNRECOVERABLE has been observed when a
        # prior process crashed mid-execution; one retry reloads the NEFF
        res = _run(in_maps)
    out1 = np.stack([res.results[b]["outT"].T for b in range(4)]).astype(np.float32)
    out2 = np.stack([res.results[4 + b]["outT"].T for b in range(4)]).astype(np.float32)
    return out1, out2



# revision 4
# speedup vs baseline: 1.0463x; 1.0463x over previous
"""Dual-path multi-head attention on 8 trn2 NeuronCores.

Sharding: core c = (path p=c//4, batch b=c%4). Each core runs the full
pipeline for one path and one batch element: 3 input projections, 16-head
attention (S=1024, dh=64), output projection. No collectives.

Path 2 cross-wiring (q2 from k; k2,v2 from q) is handled purely by host-side
input routing - every core runs the identical SPMD program.

Key speed tricks vs a plain bf16 pipeline:
- QKV projections run as fp8e4m3 DoubleRow matmuls (2 contraction rows per
  PE column pass) with a 3-pass residual decomposition
  W.x ~= Wh.xh + Wh.xr + Wr.xh (h = fp8(v), r = fp8(v - h)), all packed on
  the host. W is pre-scaled by 32 so its values sit in e4m3's normal range;
  the 32*32 factor is folded into the softmax exp scale, and 1/32 into the
  host-packed Wc. Same accuracy as bf16 at half the PE time.
- PV runs transposed: stationary = probs chunk [128k, 128q], moving =
  v1e head slot [128k, 65] -> psum [128q, 65]. Out free size 65 instead of
  512 halves PE rows; the softmax denominator rides along as a ones column
  (col 64), and normalization becomes a per-partition tensor_scalar multiply
  (no partition broadcast needed).
- The resulting [q, d] attention output is PE-transposed (128x128 blocks)
  back to [d, q] for the output projection.
- Scores psum uses a 4-bank X tile (2 key chunks -> one 2048-wide exp) plus
  a 2-bank Y tile that alternates between single-chunk scores and the next
  block's Q/K projection psum, giving 3x2048+2x1024 exp batching per head
  while fitting the 8 psum banks alongside the PV accumulators.

Emission order software-pipelines: head h emits its scores/exp interleaved
with PV+normalize of head h-1 and one Q/K projection block, so PE and Act
stay concurrently busy through the 16-head phase.
"""

import numpy as np
import ml_dtypes

B, S, D, H, DH = 4, 1024, 1024, 16, 64
NB = D // 128   # 8 partition blocks
NC2 = D // 256  # 4 pair-chunks for DoubleRow
HW = 65         # head slot width in v1e (64 data + 1 ones col)

_compiled = None


def _build():
    import concourse.bass as bass
    import concourse.mybir as mybir
    import concourse.tile as tile
    from concourse import bacc, masks

    dt = mybir.dt
    f32, bf16, f32r, fp8 = dt.float32, dt.bfloat16, dt.float32r, dt.float8e4
    DR = mybir.MatmulPerfMode.DoubleRow
    ExpF = mybir.ActivationFunctionType.Exp

    nc = bacc.Bacc("TRN2", target_bir_lowering=False, debug=False)

    xqh_d = nc.dram_tensor("xqh", [128, NC2, 2, S], fp8, kind="ExternalInput")
    xqr_d = nc.dram_tensor("xqr", [128, NC2, 2, S], fp8, kind="ExternalInput")
    xkh_d = nc.dram_tensor("xkh", [128, NC2, 2, S], fp8, kind="ExternalInput")
    xkr_d = nc.dram_tensor("xkr", [128, NC2, 2, S], fp8, kind="ExternalInput")
    xvh_d = nc.dram_tensor("xvh", [128, NC2, 2, S], fp8, kind="ExternalInput")
    xvr_d = nc.dram_tensor("xvr", [128, NC2, 2, S], fp8, kind="ExternalInput")
    wqh_d = nc.dram_tensor("wqh", [128, NB, NC2, 2, 128], fp8, kind="ExternalInput")
    wqr_d = nc.dram_tensor("wqr", [128, NB, NC2, 2, 128], fp8, kind="ExternalInput")
    wkh_d = nc.dram_tensor("wkh", [128, NB, NC2, 2, 128], fp8, kind="ExternalInput")
    wkr_d = nc.dram_tensor("wkr", [128, NB, NC2, 2, 128], fp8, kind="ExternalInput")
    wvh_d = nc.dram_tensor("wvh", [128, NC2, 2, D], fp8, kind="ExternalInput")
    wvr_d = nc.dram_tensor("wvr", [128, NC2, 2, D], fp8, kind="ExternalInput")
    wcl_d = nc.dram_tensor("wcl", [128, NB // 2, NB, 128], bf16, kind="ExternalInput")
    wch_d = nc.dram_tensor("wch", [128, NB // 2, NB, 128], bf16, kind="ExternalInput")
    bq_d = nc.dram_tensor("bq", [128, NB], f32, kind="ExternalInput")
    bk_d = nc.dram_tensor("bk", [128, NB], f32, kind="ExternalInput")
    bc_d = nc.dram_tensor("bc", [128, NB], f32, kind="ExternalInput")
    bvB_d = nc.dram_tensor("bvB", [128, D], bf16, kind="ExternalInput")
    out_d = nc.dram_tensor("outT", [D, S], f32, kind="ExternalOutput")

    ESCALE = 0.125 / 1024.0  # 1/sqrt(dh) softmax scale / (32*32 weight scale)

    with tile.TileContext(nc) as tc:
        with tc.tile_pool(name="xin", bufs=1) as xip, \
             tc.tile_pool(name="b8", bufs=2) as b8p, \
             tc.tile_pool(name="b16", bufs=2) as b16p, \
             tc.tile_pool(name="wqk", bufs=2) as wkp, \
             tc.tile_pool(name="qk", bufs=2) as qkp, \
             tc.tile_pool(name="pers", bufs=1) as prp, \
             tc.tile_pool(name="aqp", bufs=2) as aqp, \
             tc.tile_pool(name="rc", bufs=4) as rcp, \
             tc.tile_pool(name="ost", bufs=2) as ostp, \
             tc.tile_pool(name="px", bufs=1, space="PSUM") as pxp, \
             tc.tile_pool(name="py", bufs=1, space="PSUM") as pyp, \
             tc.tile_pool(name="pv", bufs=1, space="PSUM") as pvp:

            # ---- input DMAs (two queues: sync + gpsimd) ----
            xvh_t = b8p.tile([128, NC2, 2, S], fp8, tag="b8", name="xvh_t")
            nc.sync.dma_start(out=xvh_t[:, :, :, :], in_=xvh_d.ap())
            wvh_t = b16p.tile([128, NC2, 2, D], fp8, tag="b16", name="wvh_t")
            nc.gpsimd.dma_start(out=wvh_t[:, :, :, :], in_=wvh_d.ap())
            xvr_t = b8p.tile([128, NC2, 2, S], fp8, tag="b8", name="xvr_t")
            nc.sync.dma_start(out=xvr_t[:, :, :, :], in_=xvr_d.ap())
            wvr_t = b16p.tile([128, NC2, 2, D], fp8, tag="b16", name="wvr_t")
            nc.gpsimd.dma_start(out=wvr_t[:, :, :, :], in_=wvr_d.ap())

            bq_t = xip.tile([128, NB], f32, tag="bq")
            nc.sync.dma_start(out=bq_t[:, :], in_=bq_d.ap())
            bk_t = xip.tile([128, NB], f32, tag="bk")
            nc.sync.dma_start(out=bk_t[:, :], in_=bk_d.ap())
            bc_t = xip.tile([128, NB], f32, tag="bc")
            nc.sync.dma_start(out=bc_t[:, :], in_=bc_d.ap())
            bvB_t = xip.tile([128, D], bf16, tag="bv")
            nc.sync.dma_start(out=bvB_t[:, :], in_=bvB_d.ap())

            xqh_t = xip.tile([128, NC2, 2, S], fp8, tag="xqh")
            nc.gpsimd.dma_start(out=xqh_t[:, :, :, :], in_=xqh_d.ap())
            xkh_t = xip.tile([128, NC2, 2, S], fp8, tag="xkh")
            nc.sync.dma_start(out=xkh_t[:, :, :, :], in_=xkh_d.ap())
            xqr_t = xip.tile([128, NC2, 2, S], fp8, tag="xqr")
            nc.gpsimd.dma_start(out=xqr_t[:, :, :, :], in_=xqr_d.ap())
            xkr_t = xip.tile([128, NC2, 2, S], fp8, tag="xkr")
            nc.sync.dma_start(out=xkr_t[:, :, :, :], in_=xkr_d.ap())

            ident = xip.tile([128, 128], bf16, tag="id")
            masks.make_identity(nc, ident[:, :])

            # persistent tiles
            v1e = prp.tile([128, NB, H * HW], bf16)
            a1 = prp.tile([128, NB, S], bf16, tag="a1")
            ones_ap = v1e[:, :, :].rearrange("p n (h x) -> p n h x", x=HW)[:, :, :, 64]
            nc.vector.memset(ones_ap, 1.0)

            def wblk_load(w_d, m, nm, engine):
                wb = wkp.tile([128, NC2, 2, 128], fp8, tag=nm, name=f"{nm}{m}")
                engine.dma_start(out=wb[:, :, :, :], in_=w_d.ap()[:, m, :, :, :])
                return wb

            wq0h = wblk_load(wqh_d, 0, "wqh", nc.sync)
            wq0r = wblk_load(wqr_d, 0, "wqr", nc.sync)
            wk0h = wblk_load(wkh_d, 0, "wkh", nc.gpsimd)
            wk0r = wblk_load(wkr_d, 0, "wkr", nc.gpsimd)

            # ---- V projection (3-pass fp8 DoubleRow), psum alternating X/Y;
            # QK block-0 projections interleaved near the end ----
            def vproj_block(n2, pool):
                ps = pool.tile([128, 2, 512], f32, tag="s", name=f"vps{n2}")
                for half in range(2):
                    first, last = True, False
                    for xa, wa in ((xvh_t, wvh_t), (xvh_t, wvr_t), (xvr_t, wvh_t)):
                        for c in range(NC2):
                            last = xa is xvr_t and c == NC2 - 1
                            nc.tensor.matmul(
                                ps[:, half, :],
                                xa[:, c, :, n2 * 128:(n2 + 1) * 128],
                                wa[:, c, :, half * 512:(half + 1) * 512],
                                start=first, stop=last, perf_mode=DR,
                            )
                            first = False
                    dst = v1e[:, n2, :].rearrange(
                        "p (h x) -> p h x", x=HW)[:, half * 8:(half + 1) * 8, 0:64]
                    nc.vector.tensor_add(
                        dst,
                        ps[:, half, :].rearrange("p (h x) -> p h x", x=64),
                        bvB_t[:, half * 512:(half + 1) * 512].rearrange(
                            "p (h x) -> p h x", x=64))

            def proj_block(wbh, wbr, xh, xr, b_t, m, pool):
                """q1/k1 block m: [128 e', 1024 s] = 32*(W x + b), f32r."""
                ps = pool.tile([128, 2, 512], f32, tag="s", name=f"pps{m}")
                ob = qkp.tile([128, S], f32r, tag="q1" if b_t is bq_t else "k1",
                              name=f"ob{m}")
                for half in range(2):
                    first = True
                    for wa, xa in ((wbh, xh), (wbh, xr), (wbr, xh)):
                        for c in range(NC2):
                            nc.tensor.matmul(
                                ps[:, half, :],
                                wa[:, c, :, :],
                                xa[:, c, :, half * 512:(half + 1) * 512],
                                start=first,
                                stop=(wa is wbr and c == NC2 - 1),
                                perf_mode=DR,
                            )
                            first = False
                    nc.vector.tensor_scalar_add(
                        ob[:, half * 512:(half + 1) * 512], ps[:, half, :],
                        b_t[:, m:m + 1])
                return ob

            for n2 in range(5):
                vproj_block(n2, pxp if n2 % 2 == 0 else pyp)
            q1b = proj_block(wq0h, wq0r, xqh_t, xqr_t, bq_t, 0, pyp)
            vproj_block(5, pxp)
            k1b = proj_block(wk0h, wk0r, xkh_t, xkr_t, bk_t, 0, pyp)
            vproj_block(6, pxp)
            vproj_block(7, pyp)

            # wc loads reuse the xvh/xvr slots (b8 ring) once V-proj is done
            wcl_t = b8p.tile([128, NB // 2, NB, 128], bf16, tag="b8", name="wcl_t")
            nc.sync.dma_start(out=wcl_t[:, :, :, :], in_=wcl_d.ap())
            wch_t = b8p.tile([128, NB // 2, NB, 128], bf16, tag="b8", name="wch_t")
            nc.gpsimd.dma_start(out=wch_t[:, :, :, :], in_=wch_d.ap())

            # ---- attention phase ----
            state = {}  # deferred work for head h-1

            def scores_x(pt, q1b, k1b, po, n):
                """chunks n, n+1 into X, one 2048-wide exp."""
                xs = pxp.tile([128, 2, S], f32, tag="s", name=f"xs{n}")
                for j in range(2):
                    for c in range(2):
                        nc.tensor.matmul(
                            xs[:, j, c * 512:(c + 1) * 512],
                            k1b[po:po + 64, (n + j) * 128:(n + j + 1) * 128],
                            q1b[po:po + 64, c * 512:(c + 1) * 512],
                            start=True, stop=True,
                        )
                nc.scalar.activation(
                    out=pt[:, n:n + 2, :], in_=xs[:, :, :], func=ExpF, scale=ESCALE)

            def scores_y(pt, q1b, k1b, po, n):
                ys = pyp.tile([128, S], f32, tag="s", name=f"ys{n}")
                for c in range(2):
                    nc.tensor.matmul(
                        ys[:, c * 512:(c + 1) * 512],
                        k1b[po:po + 64, n * 128:(n + 1) * 128],
                        q1b[po:po + 64, c * 512:(c + 1) * 512],
                        start=True, stop=True,
                    )
                nc.scalar.activation(
                    out=pt[:, n, :], in_=ys[:, :], func=ExpF, scale=ESCALE)

            def make_pv(h, pt):
                hme = {}

                def pv_group(tag):
                    g = 0 if tag == "pva" else 1
                    ps = pvp.tile([128, 4, HW], f32, tag=tag, name=f"pv{h}{tag}")
                    hme[tag] = ps
                    for qc in range(g * 4, g * 4 + 4):
                        for n in range(NB):
                            nc.tensor.matmul(
                                ps[:, qc - g * 4, :],
                                pt[:, n, qc * 128:(qc + 1) * 128],
                                v1e[:, n, h * HW:(h + 1) * HW],
                                start=(n == 0), stop=(n == NB - 1),
                            )

                def norm(aq):
                    po = (h % 2) * 64
                    for g, tag in enumerate(("pva", "pvb")):
                        ps = hme[tag]
                        rc = rcp.tile([128, 4, 1], f32, tag="rc", name=f"rc{h}{g}")
                        nc.vector.reciprocal(rc[:, :, :], ps[:, :, 64:65])
                        for i in range(4):
                            nc.vector.tensor_scalar_mul(
                                aq[:, g * 4 + i, po:po + 64],
                                ps[:, i, 0:64], rc[:, i, :])

                return pv_group, norm

            def transpose_pair(m, aq):
                for qc in range(NB):
                    tp = pvp.tile([128, 128], bf16, tag=("pva", "pvb")[qc % 2],
                                  name=f"tp{m}{qc}")
                    nc.tensor.transpose(tp[:, :], aq[:, qc, :], ident[:, :])
                    nc.vector.tensor_copy(
                        a1[:, m, qc * 128:(qc + 1) * 128], tp[:, :])

            prev = None  # (pv_group, norm, h-1, aq tile)
            aq_cur = None
            for h in range(H):
                m = h // 2
                po = (h % 2) * 64
                pt = b16p.tile([128, NB, S], bf16, tag="b16", name=f"pt{h}")

                if h % 2 == 0:
                    aq_cur = aqp.tile([128, NB, 128], bf16, tag="aq", name=f"aq{m}")
                aq_h = aq_cur

                # stream next block's weights early (during even head)
                if h % 2 == 0 and m < NB - 1:
                    nwqh = wblk_load(wqh_d, m + 1, "wqh", nc.sync)
                    nwqr = wblk_load(wqr_d, m + 1, "wqr", nc.sync)
                    nwkh = wblk_load(wkh_d, m + 1, "wkh", nc.gpsimd)
                    nwkr = wblk_load(wkr_d, m + 1, "wkr", nc.gpsimd)

                scores_x(pt, q1b, k1b, po, 0)
                scores_y(pt, q1b, k1b, po, 2)
                scores_x(pt, q1b, k1b, po, 3)
                if prev is not None:
                    prev[0]("pva")
                if m < NB - 1:
                    if h % 2 == 0:
                        nq1b = proj_block(nwqh, nwqr, xqh_t, xqr_t, bq_t, m + 1, pyp)
                    else:
                        nk1b = proj_block(nwkh, nwkr, xkh_t, xkr_t, bk_t, m + 1, pyp)
                scores_x(pt, q1b, k1b, po, 5)
                scores_y(pt, q1b, k1b, po, 7)
                if prev is not None:
                    prev[0]("pvb")
                    prev[1](prev[3])
                    if prev[2] % 2 == 1:
                        transpose_pair(prev[2] // 2, prev[3])

                pv_group, norm = make_pv(h, pt)
                prev = (pv_group, norm, h, aq_h)
                if h % 2 == 1 and m < NB - 1:
                    q1b, k1b = nq1b, nk1b

            # finish head 15
            prev[0]("pva")
            prev[0]("pvb")
            prev[1](prev[3])
            transpose_pair(7, prev[3])

            # ---- output projection ----
            for m2 in range(NB):
                wct = wcl_t if m2 < NB // 2 else wch_t
                mi = m2 % (NB // 2)
                pool = pxp if m2 % 2 == 0 else pyp
                ops = pool.tile([128, 2, 512], f32, tag="s", name=f"ops{m2}")
                for half in range(2):
                    for n in range(NB):
                        nc.tensor.matmul(
                            ops[:, half, :], wct[:, mi, n, :],
                            a1[:, n, half * 512:(half + 1) * 512],
                            start=(n == 0), stop=(n == NB - 1),
                        )
                    ot = ostp.tile([128, 512], f32, tag="ost", name=f"ot{m2}{half}")
                    nc.vector.tensor_scalar_add(ot[:, :], ops[:, half, :],
                                                bc_t[:, m2:m2 + 1])
                    nc.sync.dma_start(
                        out=out_d.ap()[m2 * 128:(m2 + 1) * 128,
                                       half * 512:(half + 1) * 512],
                        in_=ot[:, :])

    nc.compile()
    return nc


def _get_nc():
    global _compiled
    if _compiled is None:
        _compiled = _build()
    return _compiled


def _fp8_split(a):
    e4m3 = ml_dtypes.float8_e4m3
    h = np.ascontiguousarray(a).astype(e4m3)
    r = (a - h.astype(np.float32)).astype(e4m3)
    return h, r


def _make_in_maps(q, k, v, Wq, bq, Wk, bk, Wv, bv, Wq2, bq2, Wk2, bk2, Wv2, bv2,
                  Wc, bc, Wc2, bc2):
    bf16 = ml_dtypes.bfloat16

    def xpack(x):  # [s, d] -> 2x [128, c, 2, s] fp8 (d = (2c+j)*128+p)
        xt = np.asarray(x, np.float32).T.reshape(NC2, 2, 128, S).transpose(2, 0, 1, 3)
        return _fp8_split(np.ascontiguousarray(xt))

    def wqkpack(w):  # W[e,d]*32 -> 2x [128 p, m, c, 2, e'] fp8
        wt = (32.0 * np.asarray(w, np.float32)).reshape(
            NB, 128, NC2, 2, 128).transpose(4, 0, 2, 3, 1)
        return _fp8_split(np.ascontiguousarray(wt))

    def wvpack(w):  # Wv[e,d]*32 -> 2x [128 p, c, 2, e] fp8
        wt = (32.0 * np.asarray(w, np.float32)).T.reshape(
            NC2, 2, 128, D).transpose(2, 0, 1, 3)
        return _fp8_split(np.ascontiguousarray(wt))

    def wcpack(w):  # Wc[e,d]/32 -> [128 p, m, n, e'] bf16, split in m halves
        ws = (np.asarray(w, np.float32) / 32.0).reshape(
            NB, 128, NB, 128).transpose(3, 0, 2, 1)
        ws = np.ascontiguousarray(ws).astype(bf16)
        return ws[:, :NB // 2], ws[:, NB // 2:]

    def btile(b, scale):
        return np.ascontiguousarray(
            (scale * np.asarray(b, np.float32)).reshape(NB, 128).T)

    def brep(b, scale):
        return np.ascontiguousarray(np.broadcast_to(
            scale * np.asarray(b, np.float32), (128, D))).astype(bf16)

    paths = []
    for (Wq_, bq_, Wk_, bk_, Wv_, bv_, Wc_, bc_) in (
            (Wq, bq, Wk, bk, Wv, bv, Wc, bc),
            (Wq2, bq2, Wk2, bk2, Wv2, bv2, Wc2, bc2)):
        wqh, wqr = wqkpack(Wq_)
        wkh, wkr = wqkpack(Wk_)
        wvh, wvr = wvpack(Wv_)
        wcl, wch = wcpack(Wc_)
        paths.append(dict(
            wqh=wqh, wqr=wqr, wkh=wkh, wkr=wkr, wvh=wvh, wvr=wvr,
            wcl=wcl, wch=wch,
            bq=btile(bq_, 32.0), bk=btile(bk_, 32.0), bc=btile(bc_, 1.0),
            bvB=brep(bv_, 32.0)))

    xs = {}
    for nmx, arr in (("q", q), ("k", k), ("v", v)):
        for b in range(B):
            xs[(nmx, b)] = xpack(arr[b])

    in_maps = []
    for c in range(8):
        p, b = c // 4, c % 4
        if p == 0:
            (xqh, xqr), (xkh, xkr), (xvh, xvr) = xs[("q", b)], xs[("k", b)], xs[("v", b)]
        else:
            # path 2: q2 from k; k2, v2 from q
            (xqh, xqr), (xkh, xkr), (xvh, xvr) = xs[("k", b)], xs[("q", b)], xs[("q", b)]
        in_maps.append(dict(paths[p], xqh=xqh, xqr=xqr, xkh=xkh, xkr=xkr,
                            xvh=xvh, xvr=xvr))
    return in_maps


def _run(in_maps, trace=False):
    from concourse.bass_utils import run_bass_kernel_spmd
    nc = _get_nc()
    return run_bass_kernel_spmd(nc, in_maps, core_ids=list(range(8)), trace=trace)


def kernel(**inputs):
    in_maps = _make_in_maps(**inputs)
    try:
        res = _run(in_maps)
    except Exception:
        # transient NRT_EXEC_UNIT_UNRECOVERABLE has been observed when a
        # prior process crashed mid-execution; one retry reloads the NEFF
        res = _run(in_maps)
    out1 = np.stack([res.results[b]["outT"].T for b in range(4)]).astype(np.float32)
    out2 = np.stack([res.results[4 + b]["outT"].T for b in range(4)]).astype(np.float32)
    return out1, out2


# revision 12
# speedup vs baseline: 1.0865x; 1.0385x over previous
"""Dual-path multi-head attention on 8 trn2 NeuronCores.

Sharding: core c = (path p=c//4, batch b=c%4). Each core runs the full
pipeline for one path and one batch element: 3 input projections, 16-head
attention (S=1024, dh=64), output projection. No collectives.

Path 2 cross-wiring (q2 from k; k2,v2 from q) is handled purely by host-side
input routing - every core runs the identical SPMD program.

Key speed tricks vs a plain bf16 pipeline:
- QKV projections run as fp8e4m3 DoubleRow matmuls (2 contraction rows per
  PE column pass) with a 3-pass residual decomposition
  W.x ~= Wh.xh + Wh.xr + Wr.xh (h = fp8(v), r = fp8(v - h)), all packed on
  the host. W is pre-scaled by 32 so its values sit in e4m3's normal range;
  the 32*32 factor is folded into the softmax exp scale, and 1/32 into the
  host-packed Wc. Same accuracy as bf16 at half the PE time.
- PV runs transposed: stationary = probs chunk [128k, 128q], moving =
  v1e head slot [128k, 65] -> psum [128q, 65]. Out free size 65 instead of
  512 halves PE rows; the softmax denominator rides along as a ones column
  (col 64), and normalization becomes a per-partition tensor_scalar multiply
  (no partition broadcast needed).
- The resulting [q, d] attention output is PE-transposed (128x128 blocks)
  back to [d, q] for the output projection.
- Scores psum uses a 4-bank X tile (2 key chunks -> one 2048-wide exp) plus
  a 2-bank Y tile that alternates between single-chunk scores and the next
  block's Q/K projection psum, giving 3x2048+2x1024 exp batching per head
  while fitting the 8 psum banks alongside the PV accumulators.

Emission order software-pipelines: head h emits its scores/exp interleaved
with PV+normalize of head h-1 and one Q/K projection block, so PE and Act
stay concurrently busy through the 16-head phase.
"""

import numpy as np
import ml_dtypes

B, S, D, H, DH = 4, 1024, 1024, 16, 64
NB = D // 128   # 8 partition blocks
NC2 = D // 256  # 4 pair-chunks for DoubleRow
HW = 65         # head slot width in v1e (64 data + 1 ones col)

_compiled = None


def _build():
    import concourse.bass as bass
    import concourse.mybir as mybir
    import concourse.tile as tile
    from concourse import bacc, masks

    dt = mybir.dt
    f32, bf16, f32r, fp8 = dt.float32, dt.bfloat16, dt.float32r, dt.float8e4
    DR = mybir.MatmulPerfMode.DoubleRow
    ExpF = mybir.ActivationFunctionType.Exp

    nc = bacc.Bacc("TRN2", target_bir_lowering=False, debug=False)

    xqh_d = nc.dram_tensor("xqh", [128, NC2, 2, S], fp8, kind="ExternalInput")
    xqr_d = nc.dram_tensor("xqr", [128, NC2, 2, S], fp8, kind="ExternalInput")
    xkh_d = nc.dram_tensor("xkh", [128, NC2, 2, S], fp8, kind="ExternalInput")
    xkr_d = nc.dram_tensor("xkr", [128, NC2, 2, S], fp8, kind="ExternalInput")
    xvh_d = nc.dram_tensor("xvh", [128, NC2, 2, S], fp8, kind="ExternalInput")
    xvr_d = nc.dram_tensor("xvr", [128, NC2, 2, S], fp8, kind="ExternalInput")
    wqh_d = nc.dram_tensor("wqh", [128, NB, NC2, 2, 128], fp8, kind="ExternalInput")
    wqr_d = nc.dram_tensor("wqr", [128, NB, NC2, 2, 128], fp8, kind="ExternalInput")
    wkh_d = nc.dram_tensor("wkh", [128, NB, NC2, 2, 128], fp8, kind="ExternalInput")
    wkr_d = nc.dram_tensor("wkr", [128, NB, NC2, 2, 128], fp8, kind="ExternalInput")
    wvh_d = nc.dram_tensor("wvh", [128, NC2, 2, D], fp8, kind="ExternalInput")
    wvr_d = nc.dram_tensor("wvr", [128, NC2, 2, D], fp8, kind="ExternalInput")
    wcl_d = nc.dram_tensor("wcl", [128, NB // 2, NB, 128], bf16, kind="ExternalInput")
    wch_d = nc.dram_tensor("wch", [128, NB // 2, NB, 128], bf16, kind="ExternalInput")
    bq_d = nc.dram_tensor("bq", [128, NB], f32, kind="ExternalInput")
    bk_d = nc.dram_tensor("bk", [128, NB], f32, kind="ExternalInput")
    bc_d = nc.dram_tensor("bc", [128, NB], f32, kind="ExternalInput")
    bvB_d = nc.dram_tensor("bvB", [128, D], bf16, kind="ExternalInput")
    out_d = nc.dram_tensor("outT", [D, S], bf16, kind="ExternalOutput")

    ESCALE = 0.125 / 1024.0  # 1/sqrt(dh) softmax scale / (32*32 weight scale)

    with tile.TileContext(nc) as tc:
        with tc.tile_pool(name="xin", bufs=1) as xip, \
             tc.tile_pool(name="b8", bufs=2) as b8p, \
             tc.tile_pool(name="wv", bufs=1) as wvp, \
             tc.tile_pool(name="pt", bufs=2) as ptp, \
             tc.tile_pool(name="wqk", bufs=2) as wkp, \
             tc.tile_pool(name="qk", bufs=2) as qkp, \
             tc.tile_pool(name="pers", bufs=1) as prp, \
             tc.tile_pool(name="aqp", bufs=2) as aqp, \
             tc.tile_pool(name="rc", bufs=4) as rcp, \
             tc.tile_pool(name="ost", bufs=4) as ostp, \
             tc.tile_pool(name="px", bufs=1, space="PSUM") as pxp, \
             tc.tile_pool(name="py", bufs=1, space="PSUM") as pyp, \
             tc.tile_pool(name="pj", bufs=1, space="PSUM") as pjp, \
             tc.tile_pool(name="pv", bufs=1, space="PSUM") as pvp:

            # ---- input DMAs across 4 queues (SP/HWDGE, Pool, DVE, Act) ----
            # ordering per queue puts the QK-projection inputs first so head 0
            # can start while V inputs stream in.
            bq_t = xip.tile([128, NB], f32, tag="bq")
            nc.scalar.dma_start(out=bq_t[:, :], in_=bq_d.ap())
            bk_t = xip.tile([128, NB], f32, tag="bk")
            nc.scalar.dma_start(out=bk_t[:, :], in_=bk_d.ap())
            bc_t = xip.tile([128, NB], f32, tag="bc")
            nc.scalar.dma_start(out=bc_t[:, :], in_=bc_d.ap())
            bvB_t = xip.tile([128, D], bf16, tag="bv")
            nc.scalar.dma_start(out=bvB_t[:, :], in_=bvB_d.ap())

            def wblk_load(w_d, m, nm, engine):
                wb = wkp.tile([128, NC2, 2, 128], fp8, tag=nm, name=f"{nm}{m}")
                engine.dma_start(out=wb[:, :, :, :], in_=w_d.ap()[:, m, :, :, :])
                return wb

            wq0h = wblk_load(wqh_d, 0, "wqh", nc.sync)
            wq0r = wblk_load(wqr_d, 0, "wqr", nc.sync)
            wk0h = wblk_load(wkh_d, 0, "wkh", nc.gpsimd)
            wk0r = wblk_load(wkr_d, 0, "wkr", nc.gpsimd)

            xqh_t = xip.tile([128, NC2, 2, S], fp8, tag="xqh")
            nc.gpsimd.dma_start(out=xqh_t[:, :, :, :], in_=xqh_d.ap())
            xkh_t = xip.tile([128, NC2, 2, S], fp8, tag="xkh")
            nc.sync.dma_start(out=xkh_t[:, :, :, :], in_=xkh_d.ap())
            xqr_t = xip.tile([128, NC2, 2, S], fp8, tag="xqr")
            nc.scalar.dma_start(out=xqr_t[:, :, :, :], in_=xqr_d.ap())
            xkr_t = xip.tile([128, NC2, 2, S], fp8, tag="xkr")
            nc.scalar.dma_start(out=xkr_t[:, :, :, :], in_=xkr_d.ap())

            xvh_t = b8p.tile([128, NC2, 2, S], fp8, tag="b8", name="xvh_t")
            nc.sync.dma_start(out=xvh_t[:, :, :, :], in_=xvh_d.ap())
            wvh_t = wvp.tile([128, NC2, 2, D], fp8, tag="wvh")
            nc.gpsimd.dma_start(out=wvh_t[:, :, :, :], in_=wvh_d.ap())
            xvr_t = b8p.tile([128, NC2, 2, S], fp8, tag="b8", name="xvr_t")
            nc.gpsimd.dma_start(out=xvr_t[:, :, :, :], in_=xvr_d.ap())
            wvr_t = wvp.tile([128, NC2, 2, D], fp8, tag="wvr")
            nc.sync.dma_start(out=wvr_t[:, :, :, :], in_=wvr_d.ap())

            wq1h = wblk_load(wqh_d, 1, "wqh", nc.sync)
            wq1r = wblk_load(wqr_d, 1, "wqr", nc.sync)
            wk1h = wblk_load(wkh_d, 1, "wkh", nc.gpsimd)
            wk1r = wblk_load(wkr_d, 1, "wkr", nc.gpsimd)

            ident = xip.tile([128, 128], bf16, tag="id")
            masks.make_identity(nc, ident[:, :])

            # persistent tiles
            v1e = prp.tile([128, NB, H * HW], bf16)
            a1 = prp.tile([128, NB, S], bf16, tag="a1")
            ones_ap = v1e[:, :, :].rearrange("p n (h x) -> p n h x", x=HW)[:, :, :, 64]
            nc.vector.memset(ones_ap, 1.0)

            # ---- half-granular projection emitters (fp8 DoubleRow 3-pass) ----
            def vproj_half_mm(ps_view, n2, half):
                first = True
                for xa, wa in ((xvh_t, wvh_t), (xvh_t, wvr_t), (xvr_t, wvh_t)):
                    for c in range(NC2):
                        nc.tensor.matmul(
                            ps_view,
                            xa[:, c, :, n2 * 128:(n2 + 1) * 128],
                            wa[:, c, :, half * 512:(half + 1) * 512],
                            start=first, stop=(xa is xvr_t and c == NC2 - 1),
                            perf_mode=DR,
                        )
                        first = False

            def vproj_drain(ps_view, n2, half):
                dst = v1e[:, n2, :].rearrange(
                    "p (h x) -> p h x", x=HW)[:, half * 8:(half + 1) * 8, 0:64]
                nc.vector.tensor_add(
                    dst,
                    ps_view.rearrange("p (h x) -> p h x", x=64),
                    bvB_t[:, half * 512:(half + 1) * 512].rearrange(
                        "p (h x) -> p h x", x=64))

            def vproj_block(n2, pool):
                ps = pool.tile([128, 2, 512], f32, tag="s", name=f"vps{n2}")
                for half in range(2):
                    vproj_half_mm(ps[:, half, :], n2, half)
                    vproj_drain(ps[:, half, :], n2, half)

            def vproj_half_pj(n2, half):
                ps = pjp.tile([128, 512], f32, tag="s", name=f"vpj{n2}{half}")
                vproj_half_mm(ps[:, :], n2, half)
                vproj_drain(ps[:, :], n2, half)

            def proj_half(wbh, wbr, xh, xr, b_t, ob, m, half):
                ps = pjp.tile([128, 512], f32, tag="s", name=f"pps{m}{half}")
                first = True
                for wa, xa in ((wbh, xh), (wbh, xr), (wbr, xh)):
                    for c in range(NC2):
                        nc.tensor.matmul(
                            ps[:, :], wa[:, c, :, :],
                            xa[:, c, :, half * 512:(half + 1) * 512],
                            start=first, stop=(wa is wbr and c == NC2 - 1),
                            perf_mode=DR,
                        )
                        first = False
                nc.vector.tensor_scalar_add(
                    ob[:, half * 512:(half + 1) * 512], ps[:, :], b_t[:, m:m + 1])

            def new_qk(m, which):
                return qkp.tile([128, S], f32r, tag=which, name=f"{which}_{m}")

            # ---- prologue: QK block 0 first (heads start early), then V ----
            q1b = new_qk(0, "q1")
            k1b = new_qk(0, "k1")
            proj_half(wq0h, wq0r, xqh_t, xqr_t, bq_t, q1b, 0, 0)
            proj_half(wk0h, wk0r, xkh_t, xkr_t, bk_t, k1b, 0, 0)
            proj_half(wq0h, wq0r, xqh_t, xqr_t, bq_t, q1b, 0, 1)
            proj_half(wk0h, wk0r, xkh_t, xkr_t, bk_t, k1b, 0, 1)
            for n2 in range(6):
                vproj_block(n2, pxp if n2 % 2 == 0 else pyp)
            # V blocks 6,7 run through the pj slot during head 0

            # ---- attention phase ----
            state = {}  # deferred work for head h-1

            def scores_x(pt, q1b, k1b, po, n):
                """chunks n, n+1 into X, one 2048-wide exp."""
                xs = pxp.tile([128, 2, S], f32, tag="s", name=f"xs{n}")
                for j in range(2):
                    for c in range(2):
                        nc.tensor.matmul(
                            xs[:, j, c * 512:(c + 1) * 512],
                            k1b[po:po + 64, (n + j) * 128:(n + j + 1) * 128],
                            q1b[po:po + 64, c * 512:(c + 1) * 512],
                            start=True, stop=True,
                        )
                nc.scalar.activation(
                    out=pt[:, n:n + 2, :], in_=xs[:, :, :], func=ExpF, scale=ESCALE)

            def scores_y(pt, q1b, k1b, po, n):
                ys = pyp.tile([128, S], f32, tag="s", name=f"ys{n}")
                for c in range(2):
                    nc.tensor.matmul(
                        ys[:, c * 512:(c + 1) * 512],
                        k1b[po:po + 64, n * 128:(n + 1) * 128],
                        q1b[po:po + 64, c * 512:(c + 1) * 512],
                        start=True, stop=True,
                    )
                nc.scalar.activation(
                    out=pt[:, n, :], in_=ys[:, :], func=ExpF, scale=ESCALE)

            class PVState:
                """PV + normalize for one head; emitted during the next head."""

                def __init__(self, h, pt, aq):
                    self.h, self.pt, self.aq = h, pt, aq
                    self.ps = {}

                def pv(self, tag):
                    g = 0 if tag == "pva" else 1
                    ps = pvp.tile([128, 4, HW], f32, tag="pv",
                                  name=f"pv{self.h}{tag}")
                    self.ps[tag] = ps
                    for qc in range(g * 4, g * 4 + 4):
                        for n in range(NB):
                            nc.tensor.matmul(
                                ps[:, qc - g * 4, :],
                                self.pt[:, n, qc * 128:(qc + 1) * 128],
                                v1e[:, n, self.h * HW:(self.h + 1) * HW],
                                start=(n == 0), stop=(n == NB - 1),
                            )

                def norm(self, tag):
                    g = 0 if tag == "pva" else 1
                    po = (self.h % 2) * 64
                    ps = self.ps[tag]
                    rc = rcp.tile([128, 4, 1], f32, tag="rc",
                                  name=f"rc{self.h}{g}")
                    nc.vector.reciprocal(rc[:, :, :], ps[:, :, 64:65])
                    for i in range(4):
                        nc.vector.tensor_scalar_mul(
                            self.aq[:, g * 4 + i, po:po + 64],
                            ps[:, i, 0:64], rc[:, i, :])

            def transpose_pair(m, aq):
                tp = pvp.tile([128, NB, 128], bf16, tag="pv", name=f"tp{m}")
                for qc in range(NB):
                    nc.tensor.transpose(tp[:, qc, :], aq[:, qc, :], ident[:, :])
                nc.vector.tensor_copy(
                    a1[:, m, :], tp[:, :, :].rearrange("p a b -> p (a b)"))

            qk_tiles = {0: [q1b, k1b]}
            wtiles = {1: (wq1h, wq1r, wk1h, wk1r)}
            prev = None
            aq_cur = None
            wcl_t = wch_t = None
            for h in range(H):
                m = h // 2
                po = (h % 2) * 64
                q1b, k1b = qk_tiles[m]
                pt = ptp.tile([128, NB, S], bf16, tag="pt", name=f"pt{h}")
                if h % 2 == 0:
                    aq_cur = aqp.tile([128, NB, 128], bf16, tag="aq", name=f"aq{m}")
                aq_h = aq_cur

                # pj-slot fillers for this head
                fl = []
                if h == 0:
                    fl = [(vproj_half_pj, (6, 0)), (vproj_half_pj, (6, 1)),
                          (vproj_half_pj, (7, 0)), (vproj_half_pj, (7, 1))]
                elif h == 1:
                    wqhn, wqrn, wkhn, wkrn = wtiles[1]
                    nq, nk = new_qk(1, "q1"), new_qk(1, "k1")
                    qk_tiles[1] = [nq, nk]
                    fl = [(proj_half, (wqhn, wqrn, xqh_t, xqr_t, bq_t, nq, 1, 0)),
                          (proj_half, (wqhn, wqrn, xqh_t, xqr_t, bq_t, nq, 1, 1)),
                          (proj_half, (wkhn, wkrn, xkh_t, xkr_t, bk_t, nk, 1, 0)),
                          (proj_half, (wkhn, wkrn, xkh_t, xkr_t, bk_t, nk, 1, 1))]
                elif m < NB - 1:
                    wqhn, wqrn, wkhn, wkrn = wtiles[m + 1]
                    if h % 2 == 0:
                        nq = new_qk(m + 1, "q1")
                        qk_tiles[m + 1] = [nq, None]
                        fl = [(proj_half, (wqhn, wqrn, xqh_t, xqr_t, bq_t, nq,
                                           m + 1, hf)) for hf in range(2)]
                    else:
                        nk = new_qk(m + 1, "k1")
                        qk_tiles[m + 1][1] = nk
                        fl = [(proj_half, (wkhn, wkrn, xkh_t, xkr_t, bk_t, nk,
                                           m + 1, hf)) for hf in range(2)]

                # stream weights two blocks ahead (odd heads)
                if h % 2 == 1 and m + 2 < NB:
                    wtiles[m + 2] = (wblk_load(wqh_d, m + 2, "wqh", nc.sync),
                                     wblk_load(wqr_d, m + 2, "wqr", nc.sync),
                                     wblk_load(wkh_d, m + 2, "wkh", nc.gpsimd),
                                     wblk_load(wkr_d, m + 2, "wkr", nc.gpsimd))
                if h == 1:
                    # wc loads reuse the xvh/xvr slots (b8 ring); V-proj done
                    wcl_t = b8p.tile([128, NB // 2, NB, 128], bf16, tag="b8",
                                     name="wcl_t")
                    nc.sync.dma_start(out=wcl_t[:, :, :, :], in_=wcl_d.ap())
                    wch_t = b8p.tile([128, NB // 2, NB, 128], bf16, tag="b8",
                                     name="wch_t")
                    nc.gpsimd.dma_start(out=wch_t[:, :, :, :], in_=wch_d.ap())

                if h == 0:
                    # custom pattern: py still busy with V blocks early on
                    scores_x(pt, q1b, k1b, po, 0)
                    f, a = fl.pop(0)
                    f(*a)
                    scores_x(pt, q1b, k1b, po, 2)
                    f, a = fl.pop(0)
                    f(*a)
                    scores_x(pt, q1b, k1b, po, 5)
                    scores_y(pt, q1b, k1b, po, 4)
                    f, a = fl.pop(0)
                    f(*a)
                    scores_y(pt, q1b, k1b, po, 7)
                    f, a = fl.pop(0)
                    f(*a)
                else:
                    scores_x(pt, q1b, k1b, po, 0)
                    scores_y(pt, q1b, k1b, po, 2)
                    if prev is not None:
                        prev.pv("pva")
                        prev.norm("pva")
                    scores_x(pt, q1b, k1b, po, 3)
                    if fl:
                        f, a = fl.pop(0)
                        f(*a)
                    scores_x(pt, q1b, k1b, po, 5)
                    if fl:
                        f, a = fl.pop(0)
                        f(*a)
                    scores_y(pt, q1b, k1b, po, 7)
                    if prev is not None:
                        prev.pv("pvb")
                        prev.norm("pvb")
                        if prev.h % 2 == 1:
                            transpose_pair(prev.h // 2, prev.aq)
                    while fl:
                        f, a = fl.pop(0)
                        f(*a)

                prev = PVState(h, pt, aq_h)

            # finish head 15
            prev.pv("pva")
            prev.norm("pva")
            prev.pv("pvb")
            prev.norm("pvb")
            transpose_pair(7, prev.aq)

            # ---- output projection (bf16 out, DMAs spread over 3 queues) ----
            queues = [nc.sync, nc.gpsimd, nc.scalar]
            for m2 in range(NB):
                wct = wcl_t if m2 < NB // 2 else wch_t
                mi = m2 % (NB // 2)
                pool = pxp if m2 % 2 == 0 else pyp
                ops = pool.tile([128, 2, 512], f32, tag="s", name=f"ops{m2}")
                for half in range(2):
                    for n in range(NB):
                        nc.tensor.matmul(
                            ops[:, half, :], wct[:, mi, n, :],
                            a1[:, n, half * 512:(half + 1) * 512],
                            start=(n == 0), stop=(n == NB - 1),
                        )
                    ot = ostp.tile([128, 512], bf16, tag="ost", name=f"ot{m2}{half}")
                    nc.vector.tensor_scalar_add(ot[:, :], ops[:, half, :],
                                                bc_t[:, m2:m2 + 1])
                    queues[(m2 * 2 + half) % 3].dma_start(
                        out=out_d.ap()[m2 * 128:(m2 + 1) * 128,
                                       half * 512:(half + 1) * 512],
                        in_=ot[:, :])

    nc.compile()
    return nc


def _get_nc():
    global _compiled
    if _compiled is None:
        _compiled = _build()
    return _compiled


def _fp8_split(a):
    e4m3 = ml_dtypes.float8_e4m3
    h = np.ascontiguousarray(a).astype(e4m3)
    r = (a - h.astype(np.float32)).astype(e4m3)
    return h, r


def _make_in_maps(q, k, v, Wq, bq, Wk, bk, Wv, bv, Wq2, bq2, Wk2, bk2, Wv2, bv2,
                  Wc, bc, Wc2, bc2):
    bf16 = ml_dtypes.bfloat16

    def xpack(x):  # [s, d] -> 2x [128, c, 2, s] fp8 (d = (2c+j)*128+p)
        xt = np.asarray(x, np.float32).T.reshape(NC2, 2, 128, S).transpose(2, 0, 1, 3)
        return _fp8_split(np.ascontiguousarray(xt))

    def wqkpack(w):  # W[e,d]*32 -> 2x [128 p, m, c, 2, e'] fp8
        wt = (32.0 * np.asarray(w, np.float32)).reshape(
            NB, 128, NC2, 2, 128).transpose(4, 0, 2, 3, 1)
        return _fp8_split(np.ascontiguousarray(wt))

    def wvpack(w):  # Wv[e,d]*32 -> 2x [128 p, c, 2, e] fp8
        wt = (32.0 * np.asarray(w, np.float32)).T.reshape(
            NC2, 2, 128, D).transpose(2, 0, 1, 3)
        return _fp8_split(np.ascontiguousarray(wt))

    def wcpack(w):  # Wc[e,d]/32 -> [128 p, m, n, e'] bf16, split in m halves
        ws = (np.asarray(w, np.float32) / 32.0).reshape(
            NB, 128, NB, 128).transpose(3, 0, 2, 1)
        ws = np.ascontiguousarray(ws).astype(bf16)
        return ws[:, :NB // 2], ws[:, NB // 2:]

    def btile(b, scale):
        return np.ascontiguousarray(
            (scale * np.asarray(b, np.float32)).reshape(NB, 128).T)

    def brep(b, scale):
        return np.ascontiguousarray(np.broadcast_to(
            scale * np.asarray(b, np.float32), (128, D))).astype(bf16)

    paths = []
    for (Wq_, bq_, Wk_, bk_, Wv_, bv_, Wc_, bc_) in (
            (Wq, bq, Wk, bk, Wv, bv, Wc, bc),
            (Wq2, bq2, Wk2, bk2, Wv2, bv2, Wc2, bc2)):
        wqh, wqr = wqkpack(Wq_)
        wkh, wkr = wqkpack(Wk_)
        wvh, wvr = wvpack(Wv_)
        wcl, wch = wcpack(Wc_)
        paths.append(dict(
            wqh=wqh, wqr=wqr, wkh=wkh, wkr=wkr, wvh=wvh, wvr=wvr,
            wcl=wcl, wch=wch,
            bq=btile(bq_, 32.0), bk=btile(bk_, 32.0), bc=btile(bc_, 1.0),
            bvB=brep(bv_, 32.0)))

    xs = {}
    for nmx, arr in (("q", q), ("k", k), ("v", v)):
        for b in range(B):
            xs[(nmx, b)] = xpack(arr[b])

    in_maps = []
    for c in range(8):
        p, b = c // 4, c % 4
        if p == 0:
            (xqh, xqr), (xkh, xkr), (xvh, xvr) = xs[("q", b)], xs[("k", b)], xs[("v", b)]
        else:
            # path 2: q2 from k; k2, v2 from q
            (xqh, xqr), (xkh, xkr), (xvh, xvr) = xs[("k", b)], xs[("q", b)], xs[("q", b)]
        in_maps.append(dict(paths[p], xqh=xqh, xqr=xqr, xkh=xkh, xkr=xkr,
                            xvh=xvh, xvr=xvr))
    return in_maps


def _run(in_maps, trace=False):
    from concourse.bass_utils import run_bass_kernel_spmd
    nc = _get_nc()
    return run_bass_kernel_spmd(nc, in_maps, core_ids=list(range(8)), trace=trace)


def kernel(**inputs):
    in_maps = _make_in_maps(**inputs)
    try:
        res = _run(in_maps)
    except Exception:
        # transient NRT_EXEC_UNIT_UNRECOVERABLE has been observed when a
        # prior process crashed mid-execution; one retry reloads the NEFF
        res = _run(in_maps)
    out1 = np.stack([res.results[b]["outT"].T for b in range(4)]).astype(np.float32)
    out2 = np.stack([res.results[4 + b]["outT"].T for b in range(4)]).astype(np.float32)
    return out1, out2


# revision 15
# speedup vs baseline: 1.1124x; 1.0238x over previous
"""Dual-path multi-head attention on 8 trn2 NeuronCores.

Sharding: core c = (path p=c//4, batch b=c%4). Each core runs the full
pipeline for one path and one batch element: 3 input projections, 16-head
attention (S=1024, dh=64), output projection. No collectives.

Path 2 cross-wiring (q2 from k; k2,v2 from q) is handled purely by host-side
input routing - every core runs the identical SPMD program.

Key speed tricks vs a plain bf16 pipeline:
- QKV projections run as fp8e4m3 DoubleRow matmuls (2 contraction rows per
  PE column pass) with a 3-pass residual decomposition
  W.x ~= Wh.xh + Wh.xr + Wr.xh (h = fp8(v), r = fp8(v - h)), all packed on
  the host. W is pre-scaled by 32 so its values sit in e4m3's normal range;
  the 32*32 factor is folded into the softmax exp scale, and 1/32 into the
  host-packed Wc. Same accuracy as bf16 at half the PE time.
- PV runs transposed: stationary = probs chunk [128k, 128q], moving =
  v1e head slot [128k, 65] -> psum [128q, 65]. Out free size 65 instead of
  512 halves PE rows; the softmax denominator rides along as a ones column
  (col 64), and normalization becomes a per-partition tensor_scalar multiply
  (no partition broadcast needed).
- The resulting [q, d] attention output is PE-transposed (128x128 blocks)
  back to [d, q] for the output projection.
- Scores psum uses a 4-bank X tile (2 key chunks -> one 2048-wide exp) plus
  a 2-bank Y tile that alternates between single-chunk scores and the next
  block's Q/K projection psum, giving 3x2048+2x1024 exp batching per head
  while fitting the 8 psum banks alongside the PV accumulators.

Emission order software-pipelines: head h emits its scores/exp interleaved
with PV+normalize of head h-1 and one Q/K projection block, so PE and Act
stay concurrently busy through the 16-head phase.
"""

import numpy as np
import ml_dtypes

B, S, D, H, DH = 4, 1024, 1024, 16, 64
NB = D // 128   # 8 partition blocks
NC2 = D // 256  # 4 pair-chunks for DoubleRow
HW = 65         # head slot width in v1e (64 data + 1 ones col)

_compiled = None


def _build():
    import concourse.bass as bass
    import concourse.mybir as mybir
    import concourse.tile as tile
    from concourse import bacc, masks

    dt = mybir.dt
    f32, bf16, f32r, fp8 = dt.float32, dt.bfloat16, dt.float32r, dt.float8e4
    DR = mybir.MatmulPerfMode.DoubleRow
    ExpF = mybir.ActivationFunctionType.Exp

    nc = bacc.Bacc("TRN2", target_bir_lowering=False, debug=False)

    xqh_d = nc.dram_tensor("xqh", [128, NC2, 2, S], fp8, kind="ExternalInput")
    xqr_d = nc.dram_tensor("xqr", [128, NC2, 2, S], fp8, kind="ExternalInput")
    xkh_d = nc.dram_tensor("xkh", [128, NC2, 2, S], fp8, kind="ExternalInput")
    xkr_d = nc.dram_tensor("xkr", [128, NC2, 2, S], fp8, kind="ExternalInput")
    xvh_d = nc.dram_tensor("xvh", [128, NC2, 2, S], fp8, kind="ExternalInput")
    xvr_d = nc.dram_tensor("xvr", [128, NC2, 2, S], fp8, kind="ExternalInput")
    wqh_d = nc.dram_tensor("wqh", [128, NB, NC2, 2, 128], fp8, kind="ExternalInput")
    wqr_d = nc.dram_tensor("wqr", [128, NB, NC2, 2, 128], fp8, kind="ExternalInput")
    wkh_d = nc.dram_tensor("wkh", [128, NB, NC2, 2, 128], fp8, kind="ExternalInput")
    wkr_d = nc.dram_tensor("wkr", [128, NB, NC2, 2, 128], fp8, kind="ExternalInput")
    wvh_d = nc.dram_tensor("wvh", [128, NC2, 2, D], fp8, kind="ExternalInput")
    wvr_d = nc.dram_tensor("wvr", [128, NC2, 2, D], fp8, kind="ExternalInput")
    wcl_d = nc.dram_tensor("wcl", [128, NB // 2, NB, 128], bf16, kind="ExternalInput")
    wch_d = nc.dram_tensor("wch", [128, NB // 2, NB, 128], bf16, kind="ExternalInput")
    bq_d = nc.dram_tensor("bq", [128, NB], f32, kind="ExternalInput")
    bk_d = nc.dram_tensor("bk", [128, NB], f32, kind="ExternalInput")
    bc_d = nc.dram_tensor("bc", [128, NB], f32, kind="ExternalInput")
    bvB_d = nc.dram_tensor("bvB", [128, D], bf16, kind="ExternalInput")
    out_d = nc.dram_tensor("outT", [D, S], bf16, kind="ExternalOutput")

    ESCALE = 0.125 / 1024.0  # 1/sqrt(dh) softmax scale / (32*32 weight scale)

    with tile.TileContext(nc) as tc:
        with tc.tile_pool(name="xin", bufs=1) as xip, \
             tc.tile_pool(name="b8", bufs=2) as b8p, \
             tc.tile_pool(name="wv", bufs=1) as wvp, \
             tc.tile_pool(name="pt", bufs=2) as ptp, \
             tc.tile_pool(name="wqk", bufs=2) as wkp, \
             tc.tile_pool(name="qk", bufs=2) as qkp, \
             tc.tile_pool(name="pers", bufs=1) as prp, \
             tc.tile_pool(name="aqp", bufs=2) as aqp, \
             tc.tile_pool(name="rc", bufs=4) as rcp, \
             tc.tile_pool(name="ost", bufs=4) as ostp, \
             tc.tile_pool(name="px", bufs=1, space="PSUM") as pxp, \
             tc.tile_pool(name="py", bufs=1, space="PSUM") as pyp, \
             tc.tile_pool(name="pj", bufs=1, space="PSUM") as pjp, \
             tc.tile_pool(name="pv", bufs=1, space="PSUM") as pvp:

            # ---- input DMAs: prologue-critical loads on sync+gpsimd only.
            # The scalar (Act) queue is blocked ~9us by the activation table
            # load, so it only gets loads needed later (wq1/wk1, wc, out).
            bq_t = xip.tile([128, NB], f32, tag="bq")
            nc.sync.dma_start(out=bq_t[:, :], in_=bq_d.ap())
            bk_t = xip.tile([128, NB], f32, tag="bk")
            nc.sync.dma_start(out=bk_t[:, :], in_=bk_d.ap())
            bc_t = xip.tile([128, NB], f32, tag="bc")
            nc.sync.dma_start(out=bc_t[:, :], in_=bc_d.ap())
            bvB_t = xip.tile([128, D], bf16, tag="bv")
            nc.sync.dma_start(out=bvB_t[:, :], in_=bvB_d.ap())

            def wblk_load(w_d, m, nm, engine):
                wb = wkp.tile([128, NC2, 2, 128], fp8, tag=nm, name=f"{nm}{m}")
                engine.dma_start(out=wb[:, :, :, :], in_=w_d.ap()[:, m, :, :, :])
                return wb

            wq0h = wblk_load(wqh_d, 0, "wqh", nc.sync)
            wq0r = wblk_load(wqr_d, 0, "wqr", nc.sync)
            wk0h = wblk_load(wkh_d, 0, "wkh", nc.gpsimd)
            wk0r = wblk_load(wkr_d, 0, "wkr", nc.gpsimd)

            xqh_t = xip.tile([128, NC2, 2, S], fp8, tag="xqh")
            nc.gpsimd.dma_start(out=xqh_t[:, :, :, :], in_=xqh_d.ap())
            xqr_t = xip.tile([128, NC2, 2, S], fp8, tag="xqr")
            nc.sync.dma_start(out=xqr_t[:, :, :, :], in_=xqr_d.ap())
            xkh_t = xip.tile([128, NC2, 2, S], fp8, tag="xkh")
            nc.sync.dma_start(out=xkh_t[:, :, :, :], in_=xkh_d.ap())
            xkr_t = xip.tile([128, NC2, 2, S], fp8, tag="xkr")
            nc.gpsimd.dma_start(out=xkr_t[:, :, :, :], in_=xkr_d.ap())

            xvh_t = b8p.tile([128, NC2, 2, S], fp8, tag="b8", name="xvh_t")
            nc.sync.dma_start(out=xvh_t[:, :, :, :], in_=xvh_d.ap())
            wvh_t = wvp.tile([128, NC2, 2, D], fp8, tag="wvh")
            nc.gpsimd.dma_start(out=wvh_t[:, :, :, :], in_=wvh_d.ap())
            wvr_t = wvp.tile([128, NC2, 2, D], fp8, tag="wvr")
            nc.sync.dma_start(out=wvr_t[:, :, :, :], in_=wvr_d.ap())
            xvr_t = b8p.tile([128, NC2, 2, S], fp8, tag="b8", name="xvr_t")
            nc.gpsimd.dma_start(out=xvr_t[:, :, :, :], in_=xvr_d.ap())

            wq1h = wblk_load(wqh_d, 1, "wqh", nc.scalar)
            wq1r = wblk_load(wqr_d, 1, "wqr", nc.scalar)
            wk1h = wblk_load(wkh_d, 1, "wkh", nc.scalar)
            wk1r = wblk_load(wkr_d, 1, "wkr", nc.scalar)

            ident = xip.tile([128, 128], bf16, tag="id")
            masks.make_identity(nc, ident[:, :])

            # persistent tiles
            v1e = prp.tile([128, NB, H * HW], bf16)
            a1 = prp.tile([128, NB, S], bf16, tag="a1")
            ones_ap = v1e[:, :, :].rearrange("p n (h x) -> p n h x", x=HW)[:, :, :, 64]
            nc.vector.memset(ones_ap, 1.0)

            # ---- half-granular projection emitters (fp8 DoubleRow 3-pass) ----
            def vproj_half_mm(ps_view, n2, half):
                first = True
                for xa, wa in ((xvh_t, wvh_t), (xvh_t, wvr_t), (xvr_t, wvh_t)):
                    for c in range(NC2):
                        nc.tensor.matmul(
                            ps_view,
                            xa[:, c, :, n2 * 128:(n2 + 1) * 128],
                            wa[:, c, :, half * 512:(half + 1) * 512],
                            start=first, stop=(xa is xvr_t and c == NC2 - 1),
                            perf_mode=DR,
                        )
                        first = False

            def vproj_drain(ps_view, n2, half):
                dst = v1e[:, n2, :].rearrange(
                    "p (h x) -> p h x", x=HW)[:, half * 8:(half + 1) * 8, 0:64]
                nc.vector.tensor_add(
                    dst,
                    ps_view.rearrange("p (h x) -> p h x", x=64),
                    bvB_t[:, half * 512:(half + 1) * 512].rearrange(
                        "p (h x) -> p h x", x=64))

            def vproj_block(n2, pool):
                ps = pool.tile([128, 2, 512], f32, tag="s", name=f"vps{n2}")
                for half in range(2):
                    vproj_half_mm(ps[:, half, :], n2, half)
                    vproj_drain(ps[:, half, :], n2, half)

            def vproj_half_pj(n2, half):
                ps = pjp.tile([128, 512], f32, tag="s", name=f"vpj{n2}{half}")
                vproj_half_mm(ps[:, :], n2, half)
                vproj_drain(ps[:, :], n2, half)

            def proj_half(wbh, wbr, xh, xr, b_t, ob, m, half):
                ps = pjp.tile([128, 512], f32, tag="s", name=f"pps{m}{half}")
                first = True
                for wa, xa in ((wbh, xh), (wbh, xr), (wbr, xh)):
                    for c in range(NC2):
                        nc.tensor.matmul(
                            ps[:, :], wa[:, c, :, :],
                            xa[:, c, :, half * 512:(half + 1) * 512],
                            start=first, stop=(wa is wbr and c == NC2 - 1),
                            perf_mode=DR,
                        )
                        first = False
                nc.vector.tensor_scalar_add(
                    ob[:, half * 512:(half + 1) * 512], ps[:, :], b_t[:, m:m + 1])

            def new_qk(m, which):
                return qkp.tile([128, S], f32r, tag=which, name=f"{which}_{m}")

            # ---- prologue: QK block 0 first (heads start early), then V ----
            q1b = new_qk(0, "q1")
            k1b = new_qk(0, "k1")
            proj_half(wq0h, wq0r, xqh_t, xqr_t, bq_t, q1b, 0, 0)
            proj_half(wk0h, wk0r, xkh_t, xkr_t, bk_t, k1b, 0, 0)
            proj_half(wq0h, wq0r, xqh_t, xqr_t, bq_t, q1b, 0, 1)
            proj_half(wk0h, wk0r, xkh_t, xkr_t, bk_t, k1b, 0, 1)
            for n2 in range(6):
                vproj_block(n2, pxp if n2 % 2 == 0 else pyp)
            # V blocks 6,7 run through the pj slot during head 0

            # ---- attention phase ----
            state = {}  # deferred work for head h-1

            def scores_x(pt, q1b, k1b, po, n):
                """chunks n, n+1 into X, one 2048-wide exp."""
                xs = pxp.tile([128, 2, S], f32, tag="s", name=f"xs{n}")
                for j in range(2):
                    for c in range(2):
                        nc.tensor.matmul(
                            xs[:, j, c * 512:(c + 1) * 512],
                            k1b[po:po + 64, (n + j) * 128:(n + j + 1) * 128],
                            q1b[po:po + 64, c * 512:(c + 1) * 512],
                            start=True, stop=True,
                        )
                nc.scalar.activation(
                    out=pt[:, n:n + 2, :], in_=xs[:, :, :], func=ExpF, scale=ESCALE)

            def scores_y(pt, q1b, k1b, po, n):
                ys = pyp.tile([128, S], f32, tag="s", name=f"ys{n}")
                for c in range(2):
                    nc.tensor.matmul(
                        ys[:, c * 512:(c + 1) * 512],
                        k1b[po:po + 64, n * 128:(n + 1) * 128],
                        q1b[po:po + 64, c * 512:(c + 1) * 512],
                        start=True, stop=True,
                    )
                nc.scalar.activation(
                    out=pt[:, n, :], in_=ys[:, :], func=ExpF, scale=ESCALE)

            class PVState:
                """PV + normalize for one head; emitted during the next head."""

                def __init__(self, h, pt, aq):
                    self.h, self.pt, self.aq = h, pt, aq
                    self.ps = {}

                def pv(self, tag):
                    g = 0 if tag == "pva" else 1
                    ps = pvp.tile([128, 4, HW], f32, tag="pv",
                                  name=f"pv{self.h}{tag}")
                    self.ps[tag] = ps
                    for qc in range(g * 4, g * 4 + 4):
                        for n in range(NB):
                            nc.tensor.matmul(
                                ps[:, qc - g * 4, :],
                                self.pt[:, n, qc * 128:(qc + 1) * 128],
                                v1e[:, n, self.h * HW:(self.h + 1) * HW],
                                start=(n == 0), stop=(n == NB - 1),
                            )

                def norm(self, tag):
                    g = 0 if tag == "pva" else 1
                    po = (self.h % 2) * 64
                    ps = self.ps[tag]
                    rc = rcp.tile([128, 4, 1], f32, tag="rc",
                                  name=f"rc{self.h}{g}")
                    nc.vector.reciprocal(rc[:, :, :], ps[:, :, 64:65])
                    for i in range(4):
                        nc.vector.tensor_scalar_mul(
                            self.aq[:, g * 4 + i, po:po + 64],
                            ps[:, i, 0:64], rc[:, i, :])

            def transpose_pair(m, aq):
                tp = pvp.tile([128, NB, 128], bf16, tag="pv", name=f"tp{m}")
                for qc in range(NB):
                    nc.tensor.transpose(tp[:, qc, :], aq[:, qc, :], ident[:, :])
                nc.vector.tensor_copy(
                    a1[:, m, :], tp[:, :, :].rearrange("p a b -> p (a b)"))

            qk_tiles = {0: [q1b, k1b]}
            wtiles = {1: (wq1h, wq1r, wk1h, wk1r)}
            prev = None
            aq_cur = None
            wcl_t = wch_t = None
            for h in range(H):
                m = h // 2
                po = (h % 2) * 64
                q1b, k1b = qk_tiles[m]
                pt = ptp.tile([128, NB, S], bf16, tag="pt", name=f"pt{h}")
                if h % 2 == 0:
                    aq_cur = aqp.tile([128, NB, 128], bf16, tag="aq", name=f"aq{m}")
                aq_h = aq_cur

                # pj-slot fillers for this head
                fl = []
                if h == 0:
                    fl = [(vproj_half_pj, (6, 0)), (vproj_half_pj, (6, 1)),
                          (vproj_half_pj, (7, 0)), (vproj_half_pj, (7, 1))]
                elif h == 1:
                    wqhn, wqrn, wkhn, wkrn = wtiles[1]
                    nq, nk = new_qk(1, "q1"), new_qk(1, "k1")
                    qk_tiles[1] = [nq, nk]
                    fl = [(proj_half, (wqhn, wqrn, xqh_t, xqr_t, bq_t, nq, 1, 0)),
                          (proj_half, (wqhn, wqrn, xqh_t, xqr_t, bq_t, nq, 1, 1)),
                          (proj_half, (wkhn, wkrn, xkh_t, xkr_t, bk_t, nk, 1, 0)),
                          (proj_half, (wkhn, wkrn, xkh_t, xkr_t, bk_t, nk, 1, 1))]
                elif m < NB - 1:
                    wqhn, wqrn, wkhn, wkrn = wtiles[m + 1]
                    if h % 2 == 0:
                        nq = new_qk(m + 1, "q1")
                        qk_tiles[m + 1] = [nq, None]
                        fl = [(proj_half, (wqhn, wqrn, xqh_t, xqr_t, bq_t, nq,
                                           m + 1, hf)) for hf in range(2)]
                    else:
                        nk = new_qk(m + 1, "k1")
                        qk_tiles[m + 1][1] = nk
                        fl = [(proj_half, (wkhn, wkrn, xkh_t, xkr_t, bk_t, nk,
                                           m + 1, hf)) for hf in range(2)]

                # stream weights two blocks ahead (odd heads)
                if h % 2 == 1 and m + 2 < NB:
                    wtiles[m + 2] = (wblk_load(wqh_d, m + 2, "wqh", nc.sync),
                                     wblk_load(wqr_d, m + 2, "wqr", nc.sync),
                                     wblk_load(wkh_d, m + 2, "wkh", nc.gpsimd),
                                     wblk_load(wkr_d, m + 2, "wkr", nc.gpsimd))
                if h == 1:
                    # wc loads reuse the xvh/xvr slots (b8 ring); V-proj done
                    wcl_t = b8p.tile([128, NB // 2, NB, 128], bf16, tag="b8",
                                     name="wcl_t")
                    nc.scalar.dma_start(out=wcl_t[:, :, :, :], in_=wcl_d.ap())
                    wch_t = b8p.tile([128, NB // 2, NB, 128], bf16, tag="b8",
                                     name="wch_t")
                    nc.scalar.dma_start(out=wch_t[:, :, :, :], in_=wch_d.ap())

                if h == 0:
                    # custom pattern: py still busy with V blocks early on
                    scores_x(pt, q1b, k1b, po, 0)
                    f, a = fl.pop(0)
                    f(*a)
                    scores_x(pt, q1b, k1b, po, 2)
                    f, a = fl.pop(0)
                    f(*a)
                    scores_x(pt, q1b, k1b, po, 5)
                    scores_y(pt, q1b, k1b, po, 4)
                    f, a = fl.pop(0)
                    f(*a)
                    scores_y(pt, q1b, k1b, po, 7)
                    f, a = fl.pop(0)
                    f(*a)
                else:
                    scores_x(pt, q1b, k1b, po, 0)
                    scores_y(pt, q1b, k1b, po, 2)
                    if prev is not None:
                        prev.pv("pva")
                        prev.norm("pva")
                    scores_x(pt, q1b, k1b, po, 3)
                    if fl:
                        f, a = fl.pop(0)
                        f(*a)
                    scores_x(pt, q1b, k1b, po, 5)
                    if fl:
                        f, a = fl.pop(0)
                        f(*a)
                    scores_y(pt, q1b, k1b, po, 7)
                    if prev is not None:
                        prev.pv("pvb")
                        prev.norm("pvb")
                        if prev.h % 2 == 1:
                            transpose_pair(prev.h // 2, prev.aq)
                    while fl:
                        f, a = fl.pop(0)
                        f(*a)

                prev = PVState(h, pt, aq_h)

            # finish head 15
            prev.pv("pva")
            prev.norm("pva")
            prev.pv("pvb")
            prev.norm("pvb")
            transpose_pair(7, prev.aq)

            # ---- output projection: half-granular psum rotated over all 4
            # rings (no WAR stalls), bf16 out, DMAs spread over 3 queues ----
            queues = [nc.sync, nc.gpsimd, nc.scalar]
            opools = [(pxp, "s"), (pyp, "s"), (pjp, "s"), (pvp, "pv")]
            for m2 in range(NB):
                wct = wcl_t if m2 < NB // 2 else wch_t
                mi = m2 % (NB // 2)
                for half in range(2):
                    i = m2 * 2 + half
                    pool, ptag = opools[i % 4]
                    ops = pool.tile([128, 512], f32, tag=ptag, name=f"ops{m2}{half}")
                    for n in range(NB):
                        nc.tensor.matmul(
                            ops[:, :], wct[:, mi, n, :],
                            a1[:, n, half * 512:(half + 1) * 512],
                            start=(n == 0), stop=(n == NB - 1),
                        )
                    ot = ostp.tile([128, 512], bf16, tag="ost", name=f"ot{m2}{half}")
                    nc.vector.tensor_scalar_add(ot[:, :], ops[:, :],
                                                bc_t[:, m2:m2 + 1])
                    queues[i % 3].dma_start(
                        out=out_d.ap()[m2 * 128:(m2 + 1) * 128,
                                       half * 512:(half + 1) * 512],
                        in_=ot[:, :])

    nc.compile()
    return nc


def _get_nc():
    global _compiled
    if _compiled is None:
        _compiled = _build()
    return _compiled


def _fp8_split(a):
    e4m3 = ml_dtypes.float8_e4m3
    h = np.ascontiguousarray(a).astype(e4m3)
    r = (a - h.astype(np.float32)).astype(e4m3)
    return h, r


def _make_in_maps(q, k, v, Wq, bq, Wk, bk, Wv, bv, Wq2, bq2, Wk2, bk2, Wv2, bv2,
                  Wc, bc, Wc2, bc2):
    bf16 = ml_dtypes.bfloat16

    def xpack(x):  # [s, d] -> 2x [128, c, 2, s] fp8 (d = (2c+j)*128+p)
        xt = np.asarray(x, np.float32).T.reshape(NC2, 2, 128, S).transpose(2, 0, 1, 3)
        return _fp8_split(np.ascontiguousarray(xt))

    def wqkpack(w):  # W[e,d]*32 -> 2x [128 p, m, c, 2, e'] fp8
        wt = (32.0 * np.asarray(w, np.float32)).reshape(
            NB, 128, NC2, 2, 128).transpose(4, 0, 2, 3, 1)
        return _fp8_split(np.ascontiguousarray(wt))

    def wvpack(w):  # Wv[e,d]*32 -> 2x [128 p, c, 2, e] fp8
        wt = (32.0 * np.asarray(w, np.float32)).T.reshape(
            NC2, 2, 128, D).transpose(2, 0, 1, 3)
        return _fp8_split(np.ascontiguousarray(wt))

    def wcpack(w):  # Wc[e,d]/32 -> [128 p, m, n, e'] bf16, split in m halves
        ws = (np.asarray(w, np.float32) / 32.0).reshape(
            NB, 128, NB, 128).transpose(3, 0, 2, 1)
        ws = np.ascontiguousarray(ws).astype(bf16)
        return ws[:, :NB // 2], ws[:, NB // 2:]

    def btile(b, scale):
        return np.ascontiguousarray(
            (scale * np.asarray(b, np.float32)).reshape(NB, 128).T)

    def brep(b, scale):
        return np.ascontiguousarray(np.broadcast_to(
            scale * np.asarray(b, np.float32), (128, D))).astype(bf16)

    paths = []
    for (Wq_, bq_, Wk_, bk_, Wv_, bv_, Wc_, bc_) in (
            (Wq, bq, Wk, bk, Wv, bv, Wc, bc),
            (Wq2, bq2, Wk2, bk2, Wv2, bv2, Wc2, bc2)):
        wqh, wqr = wqkpack(Wq_)
        wkh, wkr = wqkpack(Wk_)
        wvh, wvr = wvpack(Wv_)
        wcl, wch = wcpack(Wc_)
        paths.append(dict(
            wqh=wqh, wqr=wqr, wkh=wkh, wkr=wkr, wvh=wvh, wvr=wvr,
            wcl=wcl, wch=wch,
            bq=btile(bq_, 32.0), bk=btile(bk_, 32.0), bc=btile(bc_, 1.0),
            bvB=brep(bv_, 32.0)))

    xs = {}
    for nmx, arr in (("q", q), ("k", k), ("v", v)):
        for b in range(B):
            xs[(nmx, b)] = xpack(arr[b])

    in_maps = []
    for c in range(8):
        p, b = c // 4, c % 4
        if p == 0:
            (xqh, xqr), (xkh, xkr), (xvh, xvr) = xs[("q", b)], xs[("k", b)], xs[("v", b)]
        else:
            # path 2: q2 from k; k2, v2 from q
            (xqh, xqr), (xkh, xkr), (xvh, xvr) = xs[("k", b)], xs[("q", b)], xs[("q", b)]
        in_maps.append(dict(paths[p], xqh=xqh, xqr=xqr, xkh=xkh, xkr=xkr,
                            xvh=xvh, xvr=xvr))
    return in_maps


def _run(in_maps, trace=False):
    from concourse.bass_utils import run_bass_kernel_spmd
    nc = _get_nc()
    return run_bass_kernel_spmd(nc, in_maps, core_ids=list(range(8)), trace=trace)


def kernel(**inputs):
    in_maps = _make_in_maps(**inputs)
    try:
        res = _run(in_maps)
    except Exception:
        # transient NRT_EXEC_UNIT_UNRECOVERABLE has been observed when a
        # prior process crashed mid-execution; one retry reloads the NEFF
        res = _run(in_maps)
    out1 = np.stack([res.results[b]["outT"].T for b in range(4)]).astype(np.float32)
    out2 = np.stack([res.results[4 + b]["outT"].T for b in range(4)]).astype(np.float32)
    return out1, out2
